# revision 1
# baseline (speedup 1.0000x reference)
"""CRF forward-backward marginals on 8 Trainium2 NeuronCores.

Strategy (hardcoded for B=64, T=512, D=1024, K=32, 8 cores):
  - Data-parallel over batch: core i handles batches [8i, 8i+8).
  - Emissions: E'^T[k, (b,t)] = exp(x @ (W - W[:,0]) + (b - b[0])) via
    PE-transpose of x tiles + fp32r accumulate matmul + ACT Exp.
    (Subtracting the k=0 column bounds the per-(b,t) scale; marginals are
    invariant to per-(b,t) positive rescalings.)
  - Forward/backward recursions in scaled probability space with
    eUn = exp(U)/(K*e) (per-step-constant invariant):
      fwd:  p_t = (p_{t-1} @ eUn) * E'_t          p_0 = E'_0
      bwd:  w_t = (w_{t+1} @ eUn^T) * E'_t        w_{T-1} = E'_{T-1}
      marginal_t = rownorm(v_t * w_t),  v_t = p_{t-1} @ eUn  (v_0 = 1)
    Time-parallelized over 32 chunks of 16 steps with 8 burn-in steps
    (the transition kernel contracts in the Hilbert metric ~0.3x/step, so 8
    steps reach fp32 accuracy); fwd chunk 0 / bwd chunk 31 exactly re-init.
  - Combine, PE-transpose back to [t, k] layout, rownorm, DMA out.
"""

import os
import sys

import numpy as np

sys.path.insert(0, "/opt/trn_rl_repo")

import concourse.bass as bass  # noqa: E402
import concourse.bacc as bacc  # noqa: E402
import concourse.mybir as mybir  # noqa: E402
from concourse import tile  # noqa: E402
from concourse.masks import make_identity  # noqa: E402

B, T, D, K = 64, 512, 1024, 32
NCORES = 8
BL = B // NCORES            # 8 batches per core
ROWS = BL * T               # 4096 rows per core
S_CH = 16                   # chunk length
V_BI = 8                    # burn-in positions
C_CH = T // S_CH            # 32 chunks
CHAINS = BL * C_CH          # 256 parallel chains
POS = S_CH + V_BI           # 24 scan positions per direction
TP = 536                    # padded time width: [0,8) pad | t+8 | [520,536) pad
LOG_CU = -(np.log(K) + 1.0)  # log(1/(K*e)) folded into exp(U)

f32 = mybir.dt.float32
f32r = mybir.dt.float32r
AX = mybir.AxisListType
ALU = mybir.AluOpType
ACTF = mybir.ActivationFunctionType

USE_F32R = True   # reduced-precision PE matmul mode (4x faster)


def _r(ap):
    return ap.bitcast(f32r) if USE_F32R else ap


def build_nc(finalize=True):
    nc = bacc.Bacc("TRN2", target_bir_lowering=False)
    x_h = nc.declare_dram_parameter("x", [ROWS, D], f32, isOutput=False)
    w_h = nc.declare_dram_parameter("W", [D, K], f32, isOutput=False)
    u_h = nc.declare_dram_parameter("U", [K, K], f32, isOutput=False)
    b_h = nc.declare_dram_parameter("b", [1, K], f32, isOutput=False)
    o_h = nc.declare_dram_parameter("out", [ROWS, K], f32, isOutput=True)

    with tile.TileContext(nc) as tc:
        with (
            tc.tile_pool(name="const", bufs=1) as cpool,
            tc.tile_pool(name="stores", bufs=1) as spool,
        ):
            # ---------------- constants / small inputs ----------------
            id128 = cpool.tile([128, 128], f32)
            make_identity(nc, id128[:])

            w_raw = cpool.tile([128, 8, K], f32)
            nc.sync.dma_start(w_raw[:], w_h.ap().rearrange("(n p) k -> p n k", p=128))
            wn = cpool.tile([128, 8, K], f32r)
            for n in range(8):
                nc.vector.tensor_scalar_sub(wn[:, n, :], w_raw[:, n, :],
                                            w_raw[:, n, 0:1])

            u_nat = cpool.tile([K, K], f32)
            nc.sync.dma_start(u_nat[:], u_h.ap())
            u_t = cpool.tile([K, K], f32)
            nc.vector.transpose(u_t[:], u_nat[:])
            eUn = cpool.tile([K, K], f32r)
            nc.scalar.activation(eUn[:], u_nat[:], ACTF.Exp)
            eUnT = cpool.tile([K, K], f32r)
            nc.scalar.activation(eUnT[:], u_t[:], ACTF.Exp)

            b_nat = cpool.tile([1, K], f32)
            nc.sync.dma_start(b_nat[:], b_h.ap())
            one_sb = cpool.tile([1, 1], f32)
            nc.vector.memset(one_sb[:], 1.0)
            ones_row = cpool.tile([1, K], f32)
            nc.vector.memset(ones_row[:], 1.0)
            bn = cpool.tile([K, 1], f32)
            with tc.tile_pool(name="ps_b", bufs=2, space="PSUM") as ps_b_pool:
                bt_ps = ps_b_pool.tile([K, 1], f32, tag="bt")
                nc.tensor.matmul(bt_ps[:], b_nat[:], one_sb[:], start=True, stop=True)
                b0_ps = ps_b_pool.tile([K, 1], f32, tag="b0")
                nc.tensor.matmul(b0_ps[:], ones_row[:], b_nat[:, 0:1],
                                 start=True, stop=True)
                bt_sb = cpool.tile([K, 1], f32)
                nc.vector.tensor_copy(bt_sb[:], bt_ps[:])
                nc.vector.scalar_tensor_tensor(
                    bt_sb[:], bt_sb[:], float(LOG_CU), b0_ps[:],
                    op0=ALU.add, op1=ALU.subtract)
                nc.scalar.activation(bn[:], bt_sb[:], ACTF.Copy)

            # ---------------- big stores (position-major layout) ----------------
            # E2[k, b, r, c] = E'_{16c + r - 8}, r in [0,32)  (rows 0-7 / 24-31
            #   duplicate neighbor chunks so every scan read is contiguous)
            # P2 row s  = fwd state at position s   (t = 16c + s - 8)
            # W2 row 23-s = bwd state at position s (t = 16c + 23 - s)
            # V2 row u = fwd pre-multiply at t = 16c + u (u = s - 8)
            CU = float(np.exp(LOG_CU))
            E2 = spool.tile([K, BL, 32, C_CH], f32)
            P2 = spool.tile([K, BL, 24, C_CH], f32r)
            V2 = spool.tile([K, BL, 16, C_CH], f32)
            W2 = spool.tile([K, BL, 24, C_CH], f32r)
            nc.gpsimd.memset(E2[:, :, 0:8, 0], CU)
            nc.gpsimd.memset(E2[:, :, 24:32, 31], CU)

            with (
                tc.tile_pool(name="xin", bufs=2) as xpool,
                tc.tile_pool(name="xt", bufs=4) as xtpool,
                tc.tile_pool(name="outsb", bufs=3) as opool,
                tc.tile_pool(name="ps_t", bufs=2, space="PSUM") as ps_t_pool,
                tc.tile_pool(name="ps_e", bufs=1, space="PSUM") as ps_e_pool,
                tc.tile_pool(name="ps_s", bufs=3, space="PSUM") as ps_s_pool,
                tc.tile_pool(name="ps_o", bufs=2, space="PSUM") as ps_o_pool,
            ):
                # ------------- emission -------------
                for st in range(BL):
                    x_rb = []
                    for rb in range(4):
                        xt_t = xpool.tile([128, D], f32, tag=f"x{rb}")
                        nc.sync.dma_start(
                            xt_t[:],
                            x_h.ap()[st * 512 + rb * 128:st * 512 + (rb + 1) * 128, :],
                        )
                        x_rb.append(xt_t)
                    e_ps = ps_e_pool.tile([K, 512], f32, tag="eps")
                    for db in range(8):
                        ps_t = ps_t_pool.tile([128, 512], f32, tag="pst")
                        for rb in range(4):
                            nc.tensor.transpose(
                                ps_t[:, rb * 128:(rb + 1) * 128],
                                x_rb[rb][:, db * 128:(db + 1) * 128],
                                id128[:],
                            )
                        xt_sb = xtpool.tile([128, 512], f32r, tag="xt")
                        nc.vector.tensor_copy(xt_sb[:, 0:256], ps_t[:, 0:256])
                        nc.scalar.activation(xt_sb[:, 256:512], ps_t[:, 256:512],
                                             ACTF.Copy)
                        nc.tensor.matmul(
                            e_ps[:], wn[:, db, :], xt_sb[:],
                            start=(db == 0), stop=(db == 7),
                        )
                    eview = e_ps[:].rearrange("k (c u) -> k u c", u=S_CH)
                    # main rows [8, 24): t = 16c + r - 8
                    nc.scalar.activation(E2[:, st, 8:24, :], eview,
                                         ACTF.Exp, bias=bn[:, 0:1])
                    # dup rows [0, 8), c >= 1:  t = 16c - 8 + r
                    nc.scalar.activation(E2[:, st, 0:8, 1:C_CH],
                                         e_ps[:].rearrange(
                                             "k (c u) -> k u c", u=S_CH)[:, 8:16, 0:C_CH - 1],
                                         ACTF.Exp, bias=bn[:, 0:1])
                    # dup rows [24, 32), c < 31: t = 16(c+1) + r - 24
                    nc.scalar.activation(E2[:, st, 24:32, 0:C_CH - 1],
                                         e_ps[:].rearrange(
                                             "k (c u) -> k u c", u=S_CH)[:, 0:8, 1:C_CH],
                                         ACTF.Exp, bias=bn[:, 0:1])

                # ------------- scans -------------
                for s in range(POS):
                    if s == 0:
                        nc.vector.tensor_copy(P2[:, :, 0, :], E2[:, :, 0, :])
                        nc.vector.tensor_copy(W2[:, :, 23, :], E2[:, :, 31, :])
                        continue
                    psA = ps_s_pool.tile([K, CHAINS], f32, tag="ps")
                    nc.tensor.matmul(psA[:], eUn[:], P2[:, :, s - 1, :].opt(),
                                     start=True, stop=True)
                    psA3 = psA[:].rearrange("k (b c) -> k b c", b=BL)
                    nc.vector.tensor_tensor(P2[:, :, s, :], psA3, E2[:, :, s, :],
                                            op=ALU.mult)
                    if s >= V_BI:
                        nc.scalar.activation(V2[:, :, s - 8, :], psA3, ACTF.Copy)
                    psB = ps_s_pool.tile([K, CHAINS], f32, tag="ps")
                    nc.tensor.matmul(psB[:], eUnT[:], W2[:, :, 24 - s, :].opt(),
                                     start=True, stop=True)
                    psB3 = psB[:].rearrange("k (b c) -> k b c", b=BL)
                    nc.vector.tensor_tensor(W2[:, :, 23 - s, :], psB3,
                                            E2[:, :, 31 - s, :], op=ALU.mult)
                    if s == V_BI:
                        # exact re-inits once burn-in is done
                        nc.vector.tensor_copy(P2[:, :, 8, 0], E2[:, :, 8, 0])
                        nc.vector.memset(V2[:, :, 0, 0], 1.0)
                        nc.vector.tensor_copy(W2[:, :, 15, 31], E2[:, :, 23, 31])

                # ------------- combine: V2 *= W2 (pos-major, contiguous) -------------
                nc.vector.tensor_tensor(V2[:, :, 0:8, :], V2[:, :, 0:8, :],
                                        W2[:, :, 0:8, :].bitcast(f32),
                                        op=ALU.mult)
                nc.vector.tensor_tensor(V2[:, :, 8:16, :], V2[:, :, 8:16, :],
                                        W2[:, :, 8:16, :].bitcast(f32),
                                        op=ALU.mult)

                # ------------- transpose + rownorm + out -------------
                for st in range(BL):
                    ps_o = ps_o_pool.tile([128, 4, K], f32, tag="pso")
                    vflat = V2[:, st, :, :].rearrange("k u c -> k (u c)")
                    for q in range(4):
                        # partitions of ps_o[:, q, :] = (u, c) raster,
                        # u in [4q, 4q+4), all c
                        nc.tensor.transpose(ps_o[:, q, :],
                                            vflat[:, 128 * q:128 * (q + 1)],
                                            id128[:K, :K])
                    rs = opool.tile([128, 4], f32, tag="rs")
                    nc.vector.tensor_reduce(rs[:], ps_o[:], axis=AX.X, op=ALU.add)
                    rc = opool.tile([128, 4], f32, tag="rc")
                    nc.vector.reciprocal(rc[:], rs[:])
                    o_sb = opool.tile([128, 4, K], f32, tag="osb")
                    nc.vector.tensor_tensor(o_sb[:], ps_o[:],
                                            rc[:].to_broadcast((128, 4, K)),
                                            op=ALU.mult)
                    # out rows t = 16c + 4q + p1 with partition p = p1*32 + c
                    dst = o_h.ap()[st * 512:(st + 1) * 512, :].rearrange(
                        "(c q p1) k -> p1 c q k", c=32, q=4, p1=4)
                    nc.sync.dma_start(dst, o_sb[:])
    if finalize:
        nc.finalize()
    return nc


_NC_CACHE = {}


def _get_nc():
    if "nc" not in _NC_CACHE:
        _NC_CACHE["nc"] = build_nc()
    return _NC_CACHE["nc"]


def kernel(x, W, U, b):
    from concourse.bass_utils import run_bass_kernel_spmd

    nc = _get_nc()
    x = np.ascontiguousarray(np.asarray(x, np.float32))
    in_maps = [
        {
            "x": x[i * BL:(i + 1) * BL].reshape(ROWS, D),
            "W": np.asarray(W, np.float32),
            "U": np.asarray(U, np.float32),
            "b": np.asarray(b, np.float32).reshape(1, K),
        }
        for i in range(NCORES)
    ]
    res = run_bass_kernel_spmd(nc, in_maps, list(range(NCORES)),
                               trace=os.environ.get("CRF_TRACE", "") == "1")
    out = np.concatenate(
        [res.results[i]["out"].reshape(BL, T, K) for i in range(NCORES)], axis=0)
    return out


if __name__ == "__main__":
    xs = np.random.randn(B, T, D).astype(np.float32)
    Ws = (np.random.randn(D, K) / np.sqrt(D)).astype(np.float32)
    Us = (np.random.randn(K, K) * 0.1).astype(np.float32)
    bs = np.zeros(K, np.float32)
    o = kernel(xs, Ws, Us, bs)
    print(o.shape, o.dtype, o[0, 0, :4])



# revision 5
# speedup vs baseline: 1.3010x; 1.3010x over previous
"""CRF forward-backward marginals on 8 Trainium2 NeuronCores.

Strategy (hardcoded for B=64, T=512, D=1024, K=32, 8 cores):
  - Data-parallel over batch: core i handles batches [8i, 8i+8).
  - Host prep: x is transposed to [D, B_loc*T] and cast to bf16 per core,
    W is column-normalized (W - W[:,0]) and cast to bf16.  This removes all
    on-device PE transposes of x and halves the x DMA traffic.
  - Emissions: E'^T[k, (b,t)] = exp(xT^T @ Wn + bn) via bf16 streaming
    matmuls (8 d-chunks x 8 batches, N=512), overlapped with the x DMA.
    (Subtracting the k=0 column bounds the per-(b,t) scale; marginals are
    invariant to per-(b,t) positive rescalings.)
  - Forward/backward recursions in scaled probability space with
    eUn = exp(U)/(K*e) (per-step-constant invariant):
      fwd:  p_t = (p_{t-1} @ eUn) * E'_t          p_0 = E'_0
      bwd:  w_t = (w_{t+1} @ eUn^T) * E'_t        w_{T-1} = E'_{T-1}
      marginal_t = rownorm(v_t * w_t),  v_t = p_{t-1} @ eUn  (v_0 = 1)
    Time-parallelized over 32 chunks of 16 steps with 8 burn-in steps
    (the transition kernel contracts in the Hilbert metric ~0.3x/step, so 8
    steps reach fp32 accuracy); fwd chunk 0 / bwd chunk 31 exactly re-init.
  - Combine, PE-transpose back to [t, k] layout, rownorm, DMA out.
"""

import os
import sys

import numpy as np

sys.path.insert(0, "/opt/trn_rl_repo")

import concourse.bass as bass  # noqa: E402
import concourse.bacc as bacc  # noqa: E402
import concourse.mybir as mybir  # noqa: E402
from concourse import tile  # noqa: E402
from concourse.masks import make_identity  # noqa: E402

B, T, D, K = 64, 512, 1024, 32
NCORES = 8
BL = B // NCORES            # 8 batches per core
ROWS = BL * T               # 4096 rows per core
S_CH = 16                   # chunk length
V_BI = 8                    # burn-in positions
C_CH = T // S_CH            # 32 chunks
CHAINS = BL * C_CH          # 256 parallel chains
POS = S_CH + V_BI           # 24 scan positions per direction
LOG_CU = -(np.log(K) + 1.0)  # log(1/(K*e)) folded into exp(U)

f32 = mybir.dt.float32
f32r = mybir.dt.float32r
bf16 = mybir.dt.bfloat16
AX = mybir.AxisListType
ALU = mybir.AluOpType
ACTF = mybir.ActivationFunctionType

NP_BF16 = mybir.dt.np(bf16)


def build_nc(finalize=True):
    nc = bacc.Bacc("TRN2", target_bir_lowering=False)
    # x is host-transposed to [D, ROWS] and bf16; W host-normalized bf16.
    x_h = nc.declare_dram_parameter("x", [D, ROWS], bf16, isOutput=False)
    w_h = nc.declare_dram_parameter("W", [D, K], bf16, isOutput=False)
    u_h = nc.declare_dram_parameter("U", [K, K], f32, isOutput=False)
    b_h = nc.declare_dram_parameter("b", [1, K], f32, isOutput=False)
    o_h = nc.declare_dram_parameter("out", [ROWS, K], f32, isOutput=True)

    with tile.TileContext(nc) as tc:
        with (
            tc.tile_pool(name="const", bufs=1) as cpool,
            tc.tile_pool(name="stores", bufs=1) as spool,
        ):
            # ---------------- constants / small inputs ----------------
            id128 = cpool.tile([128, 128], f32)
            make_identity(nc, id128[:])

            wn = cpool.tile([128, 8, K], bf16)
            nc.sync.dma_start(wn[:], w_h.ap().rearrange("(n p) k -> p n k", p=128))

            u_nat = cpool.tile([K, K], f32)
            nc.sync.dma_start(u_nat[:], u_h.ap())
            u_t = cpool.tile([K, K], f32)
            nc.vector.transpose(u_t[:], u_nat[:])
            eUn = cpool.tile([K, K], f32r)
            nc.scalar.activation(eUn[:], u_nat[:], ACTF.Exp)
            eUnT = cpool.tile([K, K], f32r)
            nc.scalar.activation(eUnT[:], u_t[:], ACTF.Exp)

            b_nat = cpool.tile([1, K], f32)
            nc.sync.dma_start(b_nat[:], b_h.ap())
            one_sb = cpool.tile([1, 1], f32)
            nc.vector.memset(one_sb[:], 1.0)
            ones_row = cpool.tile([1, K], f32)
            nc.vector.memset(ones_row[:], 1.0)
            bn = cpool.tile([K, 1], f32)
            with tc.tile_pool(name="ps_b", bufs=2, space="PSUM") as ps_b_pool:
                bt_ps = ps_b_pool.tile([K, 1], f32, tag="bt")
                nc.tensor.matmul(bt_ps[:], b_nat[:], one_sb[:], start=True, stop=True)
                b0_ps = ps_b_pool.tile([K, 1], f32, tag="b0")
                nc.tensor.matmul(b0_ps[:], ones_row[:], b_nat[:, 0:1],
                                 start=True, stop=True)
                bt_sb = cpool.tile([K, 1], f32)
                nc.vector.tensor_copy(bt_sb[:], bt_ps[:])
                nc.vector.scalar_tensor_tensor(
                    bt_sb[:], bt_sb[:], float(LOG_CU), b0_ps[:],
                    op0=ALU.add, op1=ALU.subtract)
                nc.scalar.activation(bn[:], bt_sb[:], ACTF.Copy)

            # ---------------- big stores (position-major layout) ----------------
            # E2[k, b, r, c] = E'_{16c + r - 8}, r in [0,32)  (rows 0-7 / 24-31
            #   duplicate neighbor chunks so every scan read is contiguous)
            # P2 row s  = fwd state at position s   (t = 16c + s - 8)
            # W2 row 23-s = bwd state at position s (t = 16c + 23 - s)
            # V2 row u = fwd pre-multiply at t = 16c + u (u = s - 8)
            CU = float(np.exp(LOG_CU))
            E2 = spool.tile([K, BL, 32, C_CH], f32)
            P2 = spool.tile([K, BL, 24, C_CH], f32r)
            V2 = spool.tile([K, BL, 16, C_CH], f32)
            W2 = spool.tile([K, BL, 24, C_CH], f32r)
            nc.gpsimd.memset(E2[:, :, 0:8, 0], CU)
            nc.gpsimd.memset(E2[:, :, 24:32, 31], CU)

            # ------------- emission: e^T = Wn^T @ xT, bf16 streaming -------------
            with (
                tc.tile_pool(name="xin", bufs=1) as xpool,
                tc.tile_pool(name="ps_e", bufs=1, space="PSUM") as ps_e_pool,
            ):
                e_ps = [ps_e_pool.tile([K, 512], f32, tag=f"e{st}",
                                       name=f"e_ps{st}")
                        for st in range(BL)]
                for db in range(8):
                    xt = xpool.tile([128, ROWS], bf16, tag=f"x{db}")
                    nc.sync.dma_start(
                        xt[:], x_h.ap()[db * 128:(db + 1) * 128, :])
                    for st in range(BL):
                        nc.tensor.matmul(
                            e_ps[st][:], wn[:, db, :],
                            xt[:, st * 512:(st + 1) * 512],
                            start=(db == 0), stop=(db == 7),
                        )
                for st in range(BL):
                    eview = e_ps[st][:].rearrange("k (c u) -> k u c", u=S_CH)
                    # main rows [8, 24): t = 16c + r - 8
                    nc.scalar.activation(E2[:, st, 8:24, :], eview,
                                         ACTF.Exp, bias=bn[:, 0:1])
                    # dup rows [0, 8), c >= 1:  t = 16c - 8 + r
                    nc.scalar.activation(E2[:, st, 0:8, 1:C_CH],
                                         e_ps[st][:].rearrange(
                                             "k (c u) -> k u c", u=S_CH)[:, 8:16, 0:C_CH - 1],
                                         ACTF.Exp, bias=bn[:, 0:1])
                    # dup rows [24, 32), c < 31: t = 16(c+1) + r - 24
                    nc.scalar.activation(E2[:, st, 24:32, 0:C_CH - 1],
                                         e_ps[st][:].rearrange(
                                             "k (c u) -> k u c", u=S_CH)[:, 0:8, 1:C_CH],
                                         ACTF.Exp, bias=bn[:, 0:1])

            with (
                tc.tile_pool(name="outsb", bufs=3) as opool,
                tc.tile_pool(name="ps_s", bufs=3, space="PSUM") as ps_s_pool,
                tc.tile_pool(name="ps_o", bufs=2, space="PSUM") as ps_o_pool,
            ):
                # ------------- scans -------------
                for s in range(POS):
                    if s == 0:
                        nc.vector.tensor_copy(P2[:, :, 0, :], E2[:, :, 0, :])
                        nc.vector.tensor_copy(W2[:, :, 23, :], E2[:, :, 31, :])
                        continue
                    psA = ps_s_pool.tile([K, CHAINS], f32, tag="ps")
                    nc.tensor.matmul(psA[:], eUn[:], P2[:, :, s - 1, :].opt(),
                                     start=True, stop=True)
                    psA3 = psA[:].rearrange("k (b c) -> k b c", b=BL)
                    nc.vector.tensor_tensor(P2[:, :, s, :], psA3, E2[:, :, s, :],
                                            op=ALU.mult)
                    if s >= V_BI:
                        nc.scalar.activation(V2[:, :, s - 8, :], psA3, ACTF.Copy)
                    psB = ps_s_pool.tile([K, CHAINS], f32, tag="ps")
                    nc.tensor.matmul(psB[:], eUnT[:], W2[:, :, 24 - s, :].opt(),
                                     start=True, stop=True)
                    psB3 = psB[:].rearrange("k (b c) -> k b c", b=BL)
                    nc.vector.tensor_tensor(W2[:, :, 23 - s, :], psB3,
                                            E2[:, :, 31 - s, :], op=ALU.mult)
                    if s == V_BI:
                        # exact re-inits once burn-in is done
                        nc.vector.tensor_copy(P2[:, :, 8, 0], E2[:, :, 8, 0])
                        nc.vector.memset(V2[:, :, 0, 0], 1.0)
                        nc.vector.tensor_copy(W2[:, :, 15, 31], E2[:, :, 23, 31])

                # ------------- combine: V2 *= W2 (pos-major, contiguous) -------------
                nc.vector.tensor_tensor(V2[:, :, 0:8, :], V2[:, :, 0:8, :],
                                        W2[:, :, 0:8, :].bitcast(f32),
                                        op=ALU.mult)
                nc.vector.tensor_tensor(V2[:, :, 8:16, :], V2[:, :, 8:16, :],
                                        W2[:, :, 8:16, :].bitcast(f32),
                                        op=ALU.mult)

                # ------------- transpose + rownorm + out -------------
                for st in range(BL):
                    ps_o = ps_o_pool.tile([128, 4, K], f32, tag="pso")
                    vflat = V2[:, st, :, :].rearrange("k u c -> k (u c)")
                    for q in range(4):
                        # partitions of ps_o[:, q, :] = (u, c) raster,
                        # u in [4q, 4q+4), all c
                        nc.tensor.transpose(ps_o[:, q, :],
                                            vflat[:, 128 * q:128 * (q + 1)],
                                            id128[:K, :K])
                    rs = opool.tile([128, 4], f32, tag="rs")
                    nc.vector.tensor_reduce(rs[:], ps_o[:], axis=AX.X, op=ALU.add)
                    rc = opool.tile([128, 4], f32, tag="rc")
                    nc.vector.reciprocal(rc[:], rs[:])
                    o_sb = opool.tile([128, 4, K], f32, tag="osb")
                    nc.vector.tensor_tensor(o_sb[:], ps_o[:],
                                            rc[:].to_broadcast((128, 4, K)),
                                            op=ALU.mult)
                    # out rows t = 16c + 4q + p1 with partition p = p1*32 + c
                    dst = o_h.ap()[st * 512:(st + 1) * 512, :].rearrange(
                        "(c q p1) k -> p1 c q k", c=32, q=4, p1=4)
                    nc.sync.dma_start(dst, o_sb[:])
    if finalize:
        nc.finalize()
    return nc


_NC_CACHE = {}


def _get_nc():
    if "nc" not in _NC_CACHE:
        _NC_CACHE["nc"] = build_nc()
    return _NC_CACHE["nc"]


def _prep_host(x, W, U, b):
    """Host-side staging: shard + transpose + bf16 cast."""
    x = np.asarray(x, np.float32)
    Wn = (np.asarray(W, np.float32)
          - np.asarray(W, np.float32)[:, 0:1]).astype(NP_BF16)
    U = np.asarray(U, np.float32)
    b = np.asarray(b, np.float32).reshape(1, K)
    in_maps = []
    for i in range(NCORES):
        xT = np.ascontiguousarray(
            x[i * BL:(i + 1) * BL].reshape(ROWS, D).T).astype(NP_BF16)
        in_maps.append({"x": xT, "W": Wn, "U": U, "b": b})
    return in_maps


def kernel(x, W, U, b):
    from concourse.bass_utils import run_bass_kernel_spmd

    nc = _get_nc()
    in_maps = _prep_host(x, W, U, b)
    res = run_bass_kernel_spmd(nc, in_maps, list(range(NCORES)),
                               trace=os.environ.get("CRF_TRACE", "") == "1")
    out = np.concatenate(
        [res.results[i]["out"].reshape(BL, T, K) for i in range(NCORES)], axis=0)
    return out


if __name__ == "__main__":
    xs = np.random.randn(B, T, D).astype(np.float32)
    Ws = (np.random.randn(D, K) / np.sqrt(D)).astype(np.float32)
    Us = (np.random.randn(K, K) * 0.1).astype(np.float32)
    bs = np.zeros(K, np.float32)
    o = kernel(xs, Ws, Us, bs)
    print(o.shape, o.dtype, o[0, 0, :4])
